# revision 2
# baseline (speedup 1.0000x reference)
"""Trainium2 Bass kernel for causal multi-head attention block (nn_Attention).

Reference computation (B=4, S=2048, EMB=1024, H=16, Dh=64):
    qkv = x @ w_qkv + b_qkv ; q,k,v = split(qkv)
    out = softmax(causal(q k^T / sqrt(Dh))) @ v   (per head)
    y = merge_heads(out) @ w_fc + b_fc

Sharding: 8 cores = 4 batches x 2 head-halves (tensor parallel by head).
Core (b, hh) computes K/Q/V + attention for heads hh*8..hh*8+7 of batch b
over the FULL sequence (no K/V duplication — the big win vs data-parallel
q-splitting, which computes every batch's K/V twice).  Before the final FC
(which mixes all heads) the two cores of a batch swap attention outputs
with one 2-rank AllGather per head pair; each core then computes FC + bias
for the q-half it owns (hh=0 -> rows 0:1024, hh=1 -> rows 1024:2048).

SPMD uniformity: all 8 cores run ONE graph.  Head identity lives in
host-sliced weights; causal structure is absolute (same for both cores of
a batch); the only would-be divergence — "which q-half do I keep after the
exchange" — is handled by reading BOTH halves of the AllGather result and
blending with a host-supplied 0/1 scalar column:
    fcin = sel * slot[:, 0:1024] + (1-sel) * slot[:, 1024:2048]
wfc chunk indexing is slot-uniform (slot (p, r) -> wfc rows 512r+128p).

On-device layouts:
    xT [1024, 2048] = x[b].T (bf16, host-transposed)
    K^T, Q^T per local pair p: [128, 2048] (weight-stationary proj)
    V per kv tile st: [128 kv, 8 heads, 65] with ones-column (denominator
      falls out of the PV matmul)
    scores^T [kv, q]: per (pair, 512-q-chunk, kv tile), dual-head packed
      [128, 2x512] in one PSUM tile; exp on ScalarE (scale=1/8); diagonal
      tiles masked with a tril input AFTER exp (mask is data, not control)
    PV: nums [65, 512] per head accumulated over kv tiles
    FC: out [q, c] = sum_slots fcin_slot^T chunks @ wfc chunks

Schedule: V proj + pair-0 K/Q up front; pair p+1's K/Q projection chunks
are emitted between pair p's attention q-chunks (fills PE while ScalarE
drains exp); AllGather per pair fires as soon as that pair's attention is
normalized, overlapping the remaining pairs; FC runs after the last
exchange.
"""

import numpy as np
import ml_dtypes

B = 4
S = 2048
EMB = 1024
HEADS = 16
DH = 64
NCORES = 8
NPAIR = 4            # local head pairs per core
KV_TILES = 16        # 2048 / 128

BF16 = ml_dtypes.bfloat16

_compiled = None


def _build():
    from concourse import bacc, tile, mybir

    nc = bacc.Bacc("TRN2", target_bir_lowering=False, debug=False,
                   num_devices=NCORES)
    f32 = mybir.dt.float32
    bf16 = mybir.dt.bfloat16
    Exp = mybir.ActivationFunctionType.Exp
    Mult = mybir.AluOpType.mult
    Add = mybir.AluOpType.add

    xT = nc.dram_tensor("xT", [EMB, S], bf16, kind="ExternalInput")
    wk = nc.dram_tensor("wk", [EMB, 512], bf16, kind="ExternalInput")
    wq = nc.dram_tensor("wq", [EMB, 512], bf16, kind="ExternalInput")
    wv = nc.dram_tensor("wv", [EMB, 512], bf16, kind="ExternalInput")
    wfc = nc.dram_tensor("wfc", [EMB, EMB], bf16, kind="ExternalInput")
    bqkv_t = nc.dram_tensor("bqkv_t", [128, 8], f32, kind="ExternalInput")
    bv_bcast = nc.dram_tensor("bv_bcast", [128, 512], f32,
                              kind="ExternalInput")
    bfc_bcast = nc.dram_tensor("bfc_bcast", [128, EMB], f32,
                               kind="ExternalInput")
    mask = nc.dram_tensor("mask", [128, 256], bf16, kind="ExternalInput")
    sel = nc.dram_tensor("sel", [128, 2], f32, kind="ExternalInput")
    out = nc.dram_tensor("out", [1024, EMB], f32, kind="ExternalOutput")

    with tile.TileContext(nc) as tc:
        with (
            tc.tile_pool(name="consts", bufs=1) as consts,
            tc.tile_pool(name="vpool", bufs=1) as vpool,
            tc.tile_pool(name="xp", bufs=1) as xp,
            tc.tile_pool(name="ktpool", bufs=2) as ktpool,
            tc.tile_pool(name="qtpool", bufs=2) as qtpool,
            tc.tile_pool(name="wkp", bufs=2) as wkp,
            tc.tile_pool(name="wvp", bufs=1) as wvp,
            tc.tile_pool(name="attn", bufs=1) as attnp,
            tc.tile_pool(name="wfcp", bufs=1) as wfcp,
            tc.tile_pool(name="probs", bufs=9) as probsp,
            tc.tile_pool(name="agp", bufs=2) as agp,
            tc.tile_pool(name="fcinp", bufs=1) as fcinp,
            tc.tile_pool(name="numsb", bufs=4) as numsbp,
            tc.tile_pool(name="rbp", bufs=2) as rbp,
            tc.tile_pool(name="osb", bufs=2) as osbp,
            tc.tile_pool(name="dram", bufs=1, space="DRAM") as dram,
            tc.tile_pool(name="pwork", bufs=3, space="PSUM") as pwork,
            tc.tile_pool(name="pnum", bufs=2, space="PSUM") as pnum,
        ):
            # ---- constants (gpsimd queue: otherwise idle early) ----
            mask_sb = consts.tile([128, 256], bf16, tag="mask")
            nc.gpsimd.dma_start(out=mask_sb[:], in_=mask.ap()[:])
            bqkv_sb = consts.tile([128, 8], f32, tag="bqkv")
            nc.gpsimd.dma_start(out=bqkv_sb[:], in_=bqkv_t.ap()[:])
            bv_bc = consts.tile([128, 512], f32, tag="bvbc")
            nc.gpsimd.dma_start(out=bv_bc[:], in_=bv_bcast.ap()[:])
            bfc_bc = consts.tile([128, EMB], f32, tag="bfcbc")
            nc.gpsimd.dma_start(out=bfc_bc[:], in_=bfc_bcast.ap()[:])
            sel_sb = consts.tile([128, 2], f32, tag="sel")
            nc.gpsimd.dma_start(out=sel_sb[:], in_=sel.ap()[:])
            # warm the exp table during the projection prologue
            scratch = consts.tile([128, 8], f32, tag="scr")
            nc.scalar.activation(scratch[:], bqkv_sb[:], Exp, scale=1.0)

            # ---- bulk inputs ----
            # 2D per-e slices (128 descriptors each) — cheap to issue.
            wv_t = wvp.tile([128, 8, 512], bf16, tag="wv")
            x_sb = xp.tile([128, 8, S], bf16, tag="x")
            wfc_t = wfcp.tile([128, 8, EMB], bf16, tag="wf")

            for e in range(8):
                nc.scalar.dma_start(
                    out=wv_t[:, e, :],
                    in_=wv.ap()[128 * e:128 * e + 128, :])
            # x column-quarters split across sync/gpsimd queues
            from concourse.tile import add_dep_helper
            last_x = None
            for qq in range(4):
                eng = nc.sync if qq % 2 == 0 else nc.gpsimd
                for e in range(8):
                    last_x = eng.dma_start(
                        out=x_sb[:, e, 512 * qq:512 * qq + 512],
                        in_=xT.ap()[128 * e:128 * e + 128,
                                    512 * qq:512 * qq + 512])
            # wfc needed only at the very end; gate behind x stream
            for e in range(8):
                d = nc.scalar.dma_start(
                    out=wfc_t[:, e, :],
                    in_=wfc.ap()[128 * e:128 * e + 128, :])
                add_dep_helper(last_x.ins, d.ins, reason="gate")

            v_sb = [vpool.tile([128, 8, DH + 1], bf16, tag=f"v{st}",
                               name=f"v{st}") for st in range(KV_TILES)]
            attn_sb = [attnp.tile([128, S], bf16, tag=f"at{p}",
                                  name=f"at{p}") for p in range(NPAIR)]
            fcin = {}
            for p in range(NPAIR):
                for r in range(2):
                    fcin[(p, r)] = fcinp.tile(
                        [128, 1024], bf16, tag=f"fi{p}{r}",
                        name=f"fi{p}{r}")

            bounce_in = [dram.tile([128, 1024], bf16, tag=f"bi{p}",
                                   name=f"bi{p}") for p in range(NPAIR)]
            bounce_out = [dram.tile([256, 1024], bf16, tag=f"bo{p}",
                                    name=f"bo{p}") for p in range(NPAIR)]

            # ---- V projection (activation-stationary) ----
            def v_proj(st):
                nc.vector.memset(v_sb[st][:, :, DH:DH + 1], 1.0)
                ps = pwork.tile([128, 512], f32, tag="pw", name="pw")
                for e in range(8):
                    nc.tensor.matmul(
                        ps[:],
                        lhsT=x_sb[:, e, 128 * st:128 * st + 128],
                        rhs=wv_t[:, e, :],
                        start=(e == 0), stop=(e == 7),
                    )
                nc.vector.tensor_tensor(
                    out=v_sb[st][:, :, 0:DH], in0=ps[:], in1=bv_bc[:],
                    op=Add,
                )

            # ---- K/Q projection (weight-stationary, e-outer: 1 LDW
            # per e feeding 2 x 512-col matmuls) ----
            kq_state = {}

            def proj_pair_dma(p):
                wkt = wkp.tile([128, 8, 128], bf16, tag="wkt",
                               name=f"wkt{p}")
                wqt = wkp.tile([128, 8, 128], bf16, tag="wqt",
                               name=f"wqt{p}")
                eng = nc.scalar if p == 0 else nc.sync
                for e in range(8):
                    eng.dma_start(
                        out=wkt[:, e, :],
                        in_=wk.ap()[128 * e:128 * e + 128,
                                    128 * p:128 * p + 128])
                    eng.dma_start(
                        out=wqt[:, e, :],
                        in_=wq.ap()[128 * e:128 * e + 128,
                                    128 * p:128 * p + 128])
                kt = ktpool.tile([128, S], bf16, tag="kt", name=f"kt{p}")
                qt = qtpool.tile([128, S], bf16, tag="qt", name=f"qt{p}")
                kq_state[p] = (wkt, wqt, kt, qt)

            def proj_chunk(p, which, ci):
                """One [128, 1024] output chunk of K^T (which=0) or Q^T."""
                wkt, wqt, kt, qt = kq_state[p]
                w = wkt if which == 0 else wqt
                ps = pwork.tile([128, 1024], f32, tag="pw", name="pw")
                for e in range(8):
                    for sub in range(2):
                        nc.tensor.matmul(
                            ps[:, 512 * sub:512 * sub + 512],
                            lhsT=w[:, e, :],
                            rhs=x_sb[:, e, 1024 * ci + 512 * sub:
                                     1024 * ci + 512 * sub + 512],
                            start=(e == 0), stop=(e == 7),
                        )
                dst = kt if which == 0 else qt
                bias_col = (4 + p) if which == 0 else p
                nc.vector.tensor_scalar_add(
                    dst[:, 1024 * ci:1024 * ci + 1024], ps[:],
                    bqkv_sb[:, bias_col:bias_col + 1],
                )

            # ---- prologue: V proj + pair-0 K/Q ----
            proj_pair_dma(0)
            for st in range(8):
                v_proj(st)
            for ci in range(2):
                proj_chunk(0, 0, ci)
            for st in range(8, 12):
                v_proj(st)
            for ci in range(2):
                proj_chunk(0, 1, ci)
            for st in range(12, KV_TILES):
                v_proj(st)

            # ---- attention ----
            def normalize(p, qch, nsb):
                # nsb: both heads' [65, 512] sums already copied to SBUF
                # (frees the PSUM accumulators early; also decouples this
                # chain from the gpsimd queue, which blocks on collectives)
                r0 = rbp.tile([1, 1024], f32, tag="r0", name="r0")
                for hh2 in range(2):
                    nc.sync.dma_start(out=r0[:, 512 * hh2:512 * hh2 + 512],
                                      in_=nsb[hh2][DH:DH + 1, :])
                nc.vector.reciprocal_approx_fast(out=r0[:], in_=r0[:])
                rb = rbp.tile([DH, 1024], f32, tag="rb", name="rb")
                nc.gpsimd.partition_broadcast(rb[:], r0[:], channels=DH)
                for hh2 in range(2):
                    nc.vector.tensor_tensor(
                        out=attn_sb[p][64 * hh2:64 * hh2 + 64,
                                       512 * qch:512 * qch + 512],
                        in0=nsb[hh2][0:DH, :],
                        in1=rb[:, 512 * hh2:512 * hh2 + 512], op=Mult,
                    )

            def score_run(kt, qt, p, qch, ks, probs):
                for k in ks:
                    off = max(0, 128 * k - 512 * qch)
                    ps = pwork.tile([128, 1024], f32, tag="pw",
                                    name="pw")
                    for hh2 in range(2):
                        nc.tensor.matmul(
                            ps[:, 512 * hh2 + off:512 * hh2 + 512],
                            lhsT=kt[64 * hh2:64 * hh2 + 64,
                                    128 * k:128 * k + 128],
                            rhs=qt[64 * hh2:64 * hh2 + 64,
                                   512 * qch + off:512 * qch + 512],
                            start=True, stop=True,
                        )
                    pr = probsp.tile([128, 1024], bf16, tag="pr",
                                     name="pr")
                    probs[k] = pr
                    ps3 = ps[:].rearrange("p (a b) -> p a b", a=2)
                    pr3 = pr[:].rearrange("p (a b) -> p a b", a=2)
                    nc.scalar.activation(
                        pr3[:, :, off:512], ps3[:, :, off:512],
                        Exp, scale=0.125,
                    )
                    if 128 * k >= 512 * qch:  # diagonal tile
                        nc.vector.tensor_tensor(
                            out=pr3[:, :, off:off + 128],
                            in0=pr3[:, :, off:off + 128],
                            in1=mask_sb[:],
                            op=Mult,
                        )

            def pv_run(p, qch, ks, ntiles, probs, nums):
                for k in ks:
                    off = max(0, 128 * k - 512 * qch)
                    for hh2 in range(2):
                        nc.tensor.matmul(
                            nums[hh2][:, off:512],
                            lhsT=v_sb[k][:, 2 * p + hh2, 0:DH + 1],
                            rhs=probs[k][:, 512 * hh2 + off:
                                         512 * hh2 + 512],
                            start=(k == 0),
                            stop=(k == ntiles - 1),
                        )

            def attend_qch(p, qch):
                kt, qt = kq_state[p][2], kq_state[p][3]
                nums = [pnum.tile([DH + 1, 512], f32, tag="pn",
                                  name="pn") for _ in range(2)]
                ntiles = 4 * qch + 4
                subruns = [list(range(0, min(ntiles, 8)))]
                if ntiles > 8:
                    subruns.append(list(range(8, ntiles)))
                for ks in subruns:
                    probs = {}
                    score_run(kt, qt, p, qch, ks, probs)
                    pv_run(p, qch, ks, ntiles, probs, nums)
                nsb = []
                for hh2 in range(2):
                    t = numsbp.tile([DH + 1, 512], f32, tag="ns",
                                    name="ns")
                    nc.vector.tensor_copy(t[:], nums[hh2][:])
                    nsb.append(t)
                normalize(p, qch, nsb)

            def exchange(p):
                # send only the q-half the PARTNER owns (halves the wire):
                # snd = sel*attn[:,1024:2048] + (1-sel)*attn[:,0:1024]
                snd = agp.tile([128, 1024], bf16, tag="snd", name="snd")
                tmp0 = agp.tile([128, 1024], bf16, tag="agt", name="agt")
                nc.vector.tensor_scalar_mul(
                    tmp0[:], attn_sb[p][:, 0:1024], sel_sb[:, 1:2])
                nc.vector.scalar_tensor_tensor(
                    snd[:], attn_sb[p][:, 1024:2048], sel_sb[:, 0:1],
                    tmp0[:], Mult, Add,
                )
                nc.scalar.dma_start(out=bounce_in[p][:], in_=snd[:])
                nc.gpsimd.collective_compute(
                    "AllGather",
                    mybir.AluOpType.bypass,
                    replica_groups=[[0, 1], [2, 3], [4, 5], [6, 7]],
                    ins=[bounce_in[p].opt()],
                    outs=[bounce_out[p].opt()],
                )
                # fcin[(p,r)] = rank r's pair-p attn rows for MY q-half:
                #   r==me  -> local attn_sb half   r==partner -> AG slot r
                ag = []
                for r in range(2):
                    a = agp.tile([128, 1024], bf16, tag="ag", name="ag")
                    nc.sync.dma_start(
                        out=a[:],
                        in_=bounce_out[p][128 * r:128 * r + 128, :])
                    ag.append(a)
                # slot 0: sel -> own (attn q0) ; 1-sel -> remote slot0
                t0 = agp.tile([128, 1024], bf16, tag="agt", name="agt")
                nc.vector.tensor_scalar_mul(t0[:], ag[0][:],
                                            sel_sb[:, 1:2])
                nc.vector.scalar_tensor_tensor(
                    fcin[(p, 0)][:], attn_sb[p][:, 0:1024],
                    sel_sb[:, 0:1], t0[:], Mult, Add,
                )
                # slot 1: sel -> remote slot1 ; 1-sel -> own (attn q1)
                t1 = agp.tile([128, 1024], bf16, tag="agt", name="agt")
                nc.vector.tensor_scalar_mul(
                    t1[:], attn_sb[p][:, 1024:2048], sel_sb[:, 1:2])
                nc.vector.scalar_tensor_tensor(
                    fcin[(p, 1)][:], ag[1][:], sel_sb[:, 0:1],
                    t1[:], Mult, Add,
                )

            # proj filler pieces for pair p+1, emitted between q-chunks
            FILL = {0: (0, 0), 1: (0, 1), 2: (1, 0), 3: (1, 1)}

            for p in range(NPAIR):
                if p + 1 < NPAIR:
                    proj_pair_dma(p + 1)
                for qch in range(4):
                    attend_qch(p, qch)
                    if p + 1 < NPAIR:
                        which, ci = FILL[qch]
                        proj_chunk(p + 1, which, ci)
                del kq_state[p]
                exchange(p)

            # ---- FC ----
            def fc_qc(qc):
                ps = pwork.tile([128, 1024], f32, tag="pw", name="pw")
                ei = 0
                for pp in range(NPAIR):
                    for r in range(2):
                        we = 4 * r + pp
                        for cc in range(2):
                            nc.tensor.matmul(
                                ps[:, 512 * cc:512 * cc + 512],
                                lhsT=fcin[(pp, r)][:, 128 * qc:
                                                   128 * qc + 128],
                                rhs=wfc_t[:, we, 512 * cc:512 * cc + 512],
                                start=(ei == 0), stop=(ei == 7),
                            )
                        ei += 1
                osb = osbp.tile([128, EMB], f32, tag="ot", name="ot")
                for cc in range(2):
                    nc.vector.tensor_tensor(
                        out=osb[:, 512 * cc:512 * cc + 512],
                        in0=ps[:, 512 * cc:512 * cc + 512],
                        in1=bfc_bc[:, 512 * cc:512 * cc + 512],
                        op=Add,
                    )
                    eng = nc.sync if cc == 0 else nc.scalar
                    eng.dma_start(
                        out=out.ap()[128 * qc:128 * qc + 128,
                                     512 * cc:512 * cc + 512],
                        in_=osb[:, 512 * cc:512 * cc + 512])

            for qc in range(8):
                fc_qc(qc)

    nc.compile()
    return nc


def _get_compiled():
    global _compiled
    if _compiled is None:
        _compiled = _build()
    return _compiled


def _make_in_maps(x, w_qkv, b_qkv, w_fc, b_fc):
    wfc_bf = np.ascontiguousarray(w_fc.astype(BF16))
    bfc_bcast = np.ascontiguousarray(np.broadcast_to(
        b_fc.astype(np.float32), (128, EMB)))
    tri = np.ascontiguousarray(
        np.tril(np.ones((128, 128), dtype=np.float32)).T)  # [kv, q] kv<=q
    mask2 = np.concatenate([tri, tri], axis=1).astype(BF16)

    in_maps = []
    for core in range(NCORES):
        b, hh = core // 2, core % 2
        xTb = np.ascontiguousarray(x[b].T.astype(BF16))
        c0 = 512 * hh
        wk_s = np.ascontiguousarray(
            w_qkv[:, EMB + c0:EMB + c0 + 512].astype(BF16))
        wq_s = np.ascontiguousarray(w_qkv[:, c0:c0 + 512].astype(BF16))
        wv_s = np.ascontiguousarray(
            w_qkv[:, 2 * EMB + c0:2 * EMB + c0 + 512].astype(BF16))
        # bqkv_t[:, j] = Q bias pair j ; [:, 4+j] = K bias pair j
        bq = np.empty((128, 8), dtype=np.float32)
        for j in range(4):
            bq[:, j] = b_qkv[c0 + 128 * j:c0 + 128 * j + 128]
            bq[:, 4 + j] = b_qkv[EMB + c0 + 128 * j:EMB + c0 + 128 * j + 128]
        bv_bcast = np.ascontiguousarray(np.broadcast_to(
            b_qkv[2 * EMB + c0:2 * EMB + c0 + 512].astype(np.float32),
            (128, 512)))
        s = 1.0 if hh == 0 else 0.0
        sel = np.empty((128, 2), dtype=np.float32)
        sel[:, 0] = s
        sel[:, 1] = 1.0 - s
        in_maps.append({
            "xT": xTb,
            "wk": wk_s,
            "wq": wq_s,
            "wv": wv_s,
            "wfc": wfc_bf,
            "bqkv_t": np.ascontiguousarray(bq),
            "bv_bcast": bv_bcast,
            "bfc_bcast": bfc_bcast,
            "mask": mask2,
            "sel": sel,
        })
    return in_maps


def kernel(x, w_qkv, b_qkv, w_fc, b_fc, _trace=False):
    from concourse import bass_utils
    from concourse.bass_interp import get_hw_module

    x = np.asarray(x, dtype=np.float32)
    w_qkv = np.asarray(w_qkv, dtype=np.float32)
    b_qkv = np.asarray(b_qkv, dtype=np.float32)
    w_fc = np.asarray(w_fc, dtype=np.float32)
    b_fc = np.asarray(b_fc, dtype=np.float32)

    nc = _get_compiled()
    in_maps = _make_in_maps(x, w_qkv, b_qkv, w_fc, b_fc)

    old_m = nc.m
    nc.m = get_hw_module(nc.m)
    try:
        res = bass_utils.run_bass_kernel_spmd(
            nc, in_maps, core_ids=list(range(NCORES)), trace=_trace)
    finally:
        nc.m = old_m

    y = np.empty((B, S, EMB), dtype=np.float32)
    for core in range(NCORES):
        b, hh = core // 2, core % 2
        y[b, 1024 * hh:1024 * hh + 1024, :] = res.results[core]["out"]
    if _trace:
        kernel._last_exec_time_ns = res.exec_time_ns
        kernel._last_results = res
    return y
